# revision 56
# baseline (speedup 1.0000x reference)
"""Trainium2 Bass kernel for an AttnBlock (GroupNorm -> QKV 1x1 conv ->
spatial self-attention -> output projection -> residual).

Full-input contract: kernel(**inputs) takes the unsharded numpy inputs and
returns the full (4, 512, 64, 64) float32 output.

Sharding: 8 cores = 4 batches x 2 query-halves. Each core group-norms its
batch, runs attention for its 2048 queries over all 4096 keys, and writes
its query-half of the output. The per-core x input is column-rotated on the
host so that each core's own queries are always columns [0, 2048) — this
keeps the SPMD program identical across cores.

Algebraic fusions (all exact up to rounding):
- scores: q_i.k_j = h_j^T (Wk^T Wq) h_i + (Wk^T bq).h_j + [terms constant
  in j, dropped: softmax over j is invariant]. So K is never materialized;
  S^T = H^T @ R with R = (Wk^T Wq)^T-weighted H_q, and the (Wk^T bq).h_j
  term enters as a per-partition bias of the exp activation.
- attention output: Wp @ (V P) = (Wp Wv) @ (H P) + Wp bv (softmax weights
  sum to 1), so V is never materialized either: A = H-space attention
  (lhsT = H^T blocks), projected by M2 = Wp Wv, plus w4 = Wp bv + bp.
- softmax skips the max-subtraction (scores are ~N(0,1) after the c^-0.5
  scale); denominators: E tiles are accumulated on DVE and reduced across
  partitions by a single all-ones fp32 matmul per query chunk, and divided
  out after the output projection.

Numerics: matmuls in bf16 with fp32 PSUM accumulation; statistics, softmax
denominators and the final combine in fp32.
"""

from contextlib import ExitStack

import numpy as np

import concourse.mybir as mybir
import concourse.tile as tile
from concourse import bacc
from concourse.bass_utils import run_bass_kernel_spmd

# Problem geometry (hardcoded; the grading harness stages only kernel.py).
B = 4
C = 512
HW = 64
N = HW * HW          # 4096 keys per batch
NQ = N // 2          # 2048 queries per core
GROUPS = 32
GSIZE = C // GROUPS  # 16 channels per group
EPS = 1e-6

P = 128
CT = C // P          # 4 channel chunks
JT = N // P          # 32 key chunks of 128
NI = 512             # free-dim tile (queries / keys / channels)
IC = NQ // NI        # 4 query chunks per core

F32 = mybir.dt.float32
BF16 = mybir.dt.bfloat16

PARAM_NAMES = ("bq", "bk", "bv", "bp", "gn_scale", "gn_bias")
WEIGHT_NAMES = ("wq", "wk", "wv", "wp")

_BUILD_CACHE = {}


def _emit(ctx, nc, tc, x_d, w_d, p_d, out_d, repeat=1):
    AF = mybir.ActivationFunctionType
    ALU = mybir.AluOpType

    consts = ctx.enter_context(tc.tile_pool(name="consts", bufs=1))
    small = ctx.enter_context(tc.tile_pool(name="small", bufs=4))
    stage = ctx.enter_context(tc.tile_pool(name="stage", bufs=6))
    big = ctx.enter_context(tc.tile_pool(name="big", bufs=2))
    rpool = ctx.enter_context(tc.tile_pool(name="rpool", bufs=1))
    wpool = ctx.enter_context(tc.tile_pool(name="wpool", bufs=1))
    epool = ctx.enter_context(tc.tile_pool(name="epool", bufs=4))
    attn_pool = ctx.enter_context(tc.tile_pool(name="attn_pool", bufs=2))
    outs_pool = ctx.enter_context(tc.tile_pool(name="outs_pool", bufs=3))
    mm_ps = ctx.enter_context(tc.tile_pool(name="mm_ps", bufs=4, space="PSUM"))
    acc_ps = ctx.enter_context(tc.tile_pool(name="acc_ps", bufs=4, space="PSUM"))

    for _rep in range(repeat):
        _emit_body(nc, tc, x_d, w_d, p_d, out_d, consts, small, stage, big,
                   rpool, wpool, epool, attn_pool, outs_pool, mm_ps, acc_ps,
                   AF, ALU, _rep)


def _emit_body(nc, tc, x_d, w_d, p_d, out_d, consts, small, stage, big,
               rpool, wpool, epool, attn_pool, outs_pool, mm_ps, acc_ps,
               AF, ALU, rep):
    # ---- constants -------------------------------------------------------
    # Pool-engine constants first: the hT transposes need `ident_bf` and
    # nothing should queue ahead of it on GpSimd.
    ident_bf = consts.tile([P, P], BF16, tag="ident_bf")
    nc.gpsimd.memset(ident_bf, 0.0)
    nc.gpsimd.affine_select(
        out=ident_bf, in_=ident_bf, compare_op=ALU.not_equal, fill=1.0,
        base=0, pattern=[[-1, P]], channel_multiplier=1,
    )
    ones_f = consts.tile([P, P], F32, tag="ones_f")
    nc.vector.memset(ones_f, 1.0)

    # Per-channel params as (128, CT): column cc = channels [cc*128, ..+128).
    # SWDGE (gpsimd) keeps these small gathers off the HWDGE queues that
    # stream x and the weights.
    par = {}
    for name in PARAM_NAMES:
        t = consts.tile([P, CT], F32, tag=f"par_{name}", name=f"par_{name}")
        nc.gpsimd.dma_start(out=t, in_=p_d[name][:].rearrange("(t p) -> p t", p=P))
        par[name] = t
    bq_bf = consts.tile([P, CT], BF16, tag="bq_bf")
    nc.vector.tensor_copy(out=bq_bf, in_=par["bq"])
    bv_bf = consts.tile([P, CT], BF16, tag="bv_bf")
    nc.vector.tensor_copy(out=bv_bf, in_=par["bv"])

    # Group-reduction matrices. G: (128, 8) with G[p, g] = 1/GSIZE iff
    # p // GSIZE == g. GE: (8, 128) with GE[g, p] = 1 iff p // GSIZE == g.
    GPC = P // GSIZE  # 8 groups per 128-channel chunk
    gmat = consts.tile([P, GPC], F32, tag="gmat")
    nc.gpsimd.memset(gmat, 1.0 / GSIZE)
    nc.gpsimd.affine_select(
        out=gmat, in_=gmat, compare_op=ALU.is_ge, fill=0.0,
        base=0, pattern=[[-GSIZE, GPC]], channel_multiplier=1,
    )
    nc.gpsimd.affine_select(
        out=gmat, in_=gmat, compare_op=ALU.is_ge, fill=0.0,
        base=GSIZE - 1, pattern=[[GSIZE, GPC]], channel_multiplier=-1,
    )
    gexp = consts.tile([GPC, P], F32, tag="gexp")
    nc.gpsimd.memset(gexp, 1.0)
    nc.gpsimd.affine_select(
        out=gexp, in_=gexp, compare_op=ALU.is_ge, fill=0.0,
        base=0, pattern=[[1, P]], channel_multiplier=-GSIZE,
    )
    nc.gpsimd.affine_select(
        out=gexp, in_=gexp, compare_op=ALU.is_ge, fill=0.0,
        base=GSIZE - 1, pattern=[[-1, P]], channel_multiplier=GSIZE,
    )
    eps8 = consts.tile([GPC, 1], F32, tag="eps8")
    nc.vector.memset(eps8, EPS)

    # ---- weights: one DMA + one bf16 cast per weight --------------------
    # The host ships "wp" already transposed (c_in on rows), so all four
    # arrive in the layout their matmuls need.
    w_nat = {}
    for wname in WEIGHT_NAMES:
        w_nat[wname] = wpool.tile([P, CT, C], BF16, tag=f"wn_{wname}",
                                  name=f"wn_{wname}")
        ws = stage.tile([P, CT, C], F32, tag="wstage",
                        name=f"ws_{rep}_{wname}", bufs=2)
        nc.sync.dma_start(
            out=ws, in_=w_d[wname][:].rearrange("(t p) c -> p t c", p=P))
        nc.vector.tensor_copy(out=w_nat[wname], in_=ws)
    wpT = w_nat["wp"]

    # ---- weight-only fusions (overlap with the x DMA / GroupNorm) --------
    # W3 = Wq^T Wk, stored (b=c_q partition-chunks, a=c_k free).
    w3 = wpool.tile([P, CT, C], BF16, tag="w3")
    for bt in range(CT):
        ps = mm_ps.tile([P, C], F32, tag="mm")
        for co in range(CT):
            nc.tensor.matmul(
                ps, lhsT=w_nat["wq"][:, co, bt * P:(bt + 1) * P],
                rhs=w_nat["wk"][:, co, :],
                start=(co == 0), stop=(co == CT - 1))
        nc.vector.tensor_copy(out=w3[:, bt, :], in_=ps)
    # M2T = (Wp Wv)^T, stored (a=c_attn partition-chunks, d=c_out free).
    m2t = wpool.tile([P, CT, C], BF16, tag="m2t")
    for at in range(CT):
        ps = mm_ps.tile([P, C], F32, tag="mm")
        for ec in range(CT):
            nc.tensor.matmul(
                ps, lhsT=w_nat["wv"][:, ec, at * P:(at + 1) * P],
                rhs=wpT[:, ec, :],
                start=(ec == 0), stop=(ec == CT - 1))
        nc.vector.tensor_copy(out=m2t[:, at, :], in_=ps)
    # w2 = Wk^T bq (bf16, used as a matmul operand against h).
    w2_bf = consts.tile([P, CT], BF16, tag="w2_bf")
    for at in range(CT):
        ps = mm_ps.tile([P, 1], F32, tag="mm")
        for co in range(CT):
            nc.tensor.matmul(
                ps, lhsT=w_nat["wk"][:, co, at * P:(at + 1) * P],
                rhs=bq_bf[:, co:co + 1],
                start=(co == 0), stop=(co == CT - 1))
        nc.vector.tensor_copy(out=w2_bf[:, at:at + 1], in_=ps)
    # w4 = Wp bv + bp (per output channel, f32).
    w4 = consts.tile([P, CT], F32, tag="w4")
    for dt_ in range(CT):
        ps = mm_ps.tile([P, 1], F32, tag="mm")
        for ec in range(CT):
            nc.tensor.matmul(
                ps, lhsT=wpT[:, ec, dt_ * P:(dt_ + 1) * P],
                rhs=bv_bf[:, ec:ec + 1],
                start=(ec == 0), stop=(ec == CT - 1))
        nc.vector.tensor_add(out=w4[:, dt_:dt_ + 1], in0=ps,
                             in1=par["bp"][:, dt_:dt_ + 1])

    # ---- x load + GroupNorm + normalize (to bf16 h) ----------------------
    h = big.tile([P, CT, N], BF16, tag="big")
    # hT blocks (keys on partitions), filled per channel chunk as h lands.
    ht = big.tile([P, JT, C], BF16, tag="big")
    for cc in range(CT):
        stats = small.tile([P, 8, 6], F32, tag="gn_stats",
                           name=f"gn_stats_{rep}_{cc}")
        xs = stage.tile([P, N], F32, tag="xstage", name=f"xs_{rep}_{cc}",
                        bufs=2)
        nc.sync.dma_start(out=xs, in_=x_d[cc * P:(cc + 1) * P, :])
        for sg in range(8):
            nc.vector.bn_stats(out=stats[:, sg, :],
                               in_=xs[:, sg * NI:(sg + 1) * NI])
        mv = small.tile([P, 2], F32, tag="gn_mv")
        nc.vector.bn_aggr(out=mv, in_=stats)
        # stat2 = [mean_c, E[x^2]_c];  E[x^2] = mean^2 + var in one op
        stat2 = small.tile([P, 2], F32, tag="gn_stat2")
        nc.vector.tensor_copy(out=stat2[:, 0:1], in_=mv[:, 0:1])
        nc.vector.tensor_scalar(
            out=stat2[:, 1:2], in0=mv[:, 0:1], scalar1=mv[:, 0:1],
            scalar2=mv[:, 1:2], op0=ALU.mult, op1=ALU.add)
        # group-combine on PE: (8, 2) = G^T @ stat2
        g_ps = acc_ps.tile([GPC, 2], F32, tag="acc")
        nc.tensor.matmul(g_ps, lhsT=gmat, rhs=stat2, start=True, stop=True)
        g_sb = small.tile([GPC, 2], F32, tag="gn_gsb")
        nc.vector.tensor_copy(out=g_sb, in_=g_ps)
        # grp = [mean_g, rstd_g];  rstd via sqrt(-1*(mean^2 - E2) + eps)
        grp = small.tile([GPC, 2], F32, tag="gn_grp")
        nc.vector.tensor_copy(out=grp[:, 0:1], in_=g_sb[:, 0:1])
        nvar = small.tile([GPC, 1], F32, tag="gn_nvar")
        nc.vector.tensor_scalar(
            out=nvar, in0=g_sb[:, 0:1], scalar1=g_sb[:, 0:1],
            scalar2=g_sb[:, 1:2], op0=ALU.mult, op1=ALU.subtract)
        sd = small.tile([GPC, 1], F32, tag="gn_sd")
        nc.scalar.activation(out=sd, in_=nvar, func=AF.Sqrt, bias=eps8,
                             scale=-1.0)
        nc.vector.reciprocal(out=grp[:, 1:2], in_=sd)
        # expand back to per-channel via PE: (128, 2) = GE^T @ grp
        e_ps = acc_ps.tile([P, 2], F32, tag="acc")
        nc.tensor.matmul(e_ps, lhsT=gexp, rhs=grp, start=True, stop=True)
        e_sb = small.tile([P, 2], F32, tag="gn_esb")
        nc.vector.tensor_copy(out=e_sb, in_=e_ps)
        # a_c = gn_scale * rstd ; b_c = gn_bias - mean * a_c
        a_c = small.tile([P, 1], F32, tag="gn_a")
        nc.vector.tensor_mul(out=a_c, in0=par["gn_scale"][:, cc:cc + 1],
                             in1=e_sb[:, 1:2])
        nb_c = small.tile([P, 1], F32, tag="gn_nb")
        nc.vector.tensor_scalar(
            out=nb_c, in0=e_sb[:, 0:1], scalar1=a_c,
            scalar2=par["gn_bias"][:, cc:cc + 1],
            op0=ALU.mult, op1=ALU.subtract)
        # h = a_c * x - nb_c   (f32 staging in, bf16 out)
        nc.vector.tensor_scalar(
            out=h[:, cc, :], in0=xs, scalar1=a_c, scalar2=nb_c,
            op0=ALU.mult, op1=ALU.subtract)
        # hT blocks for this channel chunk: 4 transposes packed per PSUM
        # bank (disjoint column ranges), one strided eviction per pack.
        for jg in range(JT // 4):
            tp = acc_ps.tile([P, 4, P], BF16, tag="acc",
                             name=f"htp_{rep}_{cc}_{jg}")
            for k in range(4):
                jc = jg * 4 + k
                nc.tensor.matmul(
                    tp[:, k, :], lhsT=h[:, cc, jc * P:(jc + 1) * P],
                    rhs=ident_bf, is_transpose=True, skip_group_check=True)
            nc.vector.tensor_copy(
                out=ht[:, jg * 4:(jg + 1) * 4, cc * P:(cc + 1) * P], in_=tp)

    # ---- h-derived operands ---------------------------------------------
    inv_sqrt_c = float(C) ** -0.5
    # R = (Wk^T Wq)^T-weighted H_q: R[a, i] = sum_b W3[b, a] h[b, i].
    # icq-major so attention on the first query chunk can start early.
    r_sb = rpool.tile([P, CT, NQ], BF16, tag="r")
    for icq in range(IC):
        for at in range(CT):
            ps = mm_ps.tile([P, NI], F32, tag="mm")
            for bc in range(CT):
                nc.tensor.matmul(
                    ps, lhsT=w3[:, bc, at * P:(at + 1) * P],
                    rhs=h[:, bc, icq * NI:(icq + 1) * NI],
                    start=(bc == 0), stop=(bc == CT - 1))
            nc.vector.tensor_copy(out=r_sb[:, at, icq * NI:(icq + 1) * NI],
                                  in_=ps)
    # r2[j] = (Wk^T bq) . h_j, scaled by c^-0.5: per-partition exp bias.
    # 8 j-chunks pack into one PSUM bank (disjoint f32 columns).
    r2s = consts.tile([P, JT], F32, tag="r2s")
    for jg in range(JT // 8):
        ps = acc_ps.tile([P, 8], F32, tag="acc", name=f"r2p_{rep}_{jg}")
        for k in range(8):
            jc = jg * 8 + k
            for ac in range(CT):
                nc.tensor.matmul(
                    ps[:, k:k + 1], lhsT=h[:, ac, jc * P:(jc + 1) * P],
                    rhs=w2_bf[:, ac:ac + 1],
                    start=(ac == 0), stop=(ac == CT - 1),
                    skip_group_check=True)
        nc.vector.tensor_scalar_mul(out=r2s[:, jg * 8:(jg + 1) * 8], in0=ps,
                                    scalar1=inv_sqrt_c)

    # ---- attention + output projection + residual ------------------------
    for icq in range(IC):
        att_ps = [acc_ps.tile([P, NI], F32, tag="acc",
                              name=f"att_ps_{rep}_{icq}_{ct}")
                  for ct in range(CT)]
        e_sum = outs_pool.tile([P, NI], F32, tag="esum", bufs=2,
                                name=f"esum_{rep}_{icq}")
        for jc in range(JT):
            s_ps = mm_ps.tile([P, NI], F32, tag="mm")
            for ac in range(CT):
                nc.tensor.matmul(
                    s_ps, lhsT=h[:, ac, jc * P:(jc + 1) * P],
                    rhs=r_sb[:, ac, icq * NI:(icq + 1) * NI],
                    start=(ac == 0), stop=(ac == CT - 1))
            e = epool.tile([P, NI], BF16, tag="e")
            nc.scalar.activation(out=e, in_=s_ps, func=AF.Exp,
                                 scale=inv_sqrt_c, bias=r2s[:, jc:jc + 1])
            for ct in range(CT):
                nc.tensor.matmul(
                    att_ps[ct], lhsT=ht[:, jc, ct * P:(ct + 1) * P], rhs=e,
                    start=(jc == 0), stop=(jc == JT - 1))
            if jc == 0:
                nc.vector.tensor_copy(out=e_sum, in_=e)
            else:
                nc.vector.tensor_add(out=e_sum, in0=e_sum, in1=e)
        den_ps = mm_ps.tile([P, NI], F32, tag="mm",
                            name=f"den_ps_{rep}_{icq}")
        nc.tensor.matmul(den_ps, lhsT=ones_f, rhs=e_sum, start=True, stop=True)
        rec = outs_pool.tile([P, NI], F32, tag="rec", bufs=2,
                              name=f"rec_{rep}_{icq}")
        nc.vector.reciprocal(out=rec, in_=den_ps)
        att_sb = attn_pool.tile([P, CT, NI], BF16, tag="attn")
        for ct in range(CT):
            nc.vector.tensor_copy(out=att_sb[:, ct, :], in_=att_ps[ct])
        xr = outs_pool.tile([P, CT, NI], F32, tag="xres", bufs=2,
                            name=f"xr_{rep}_{icq}")
        nc.sync.dma_start(
            out=xr, in_=x_d[:, icq * NI:(icq + 1) * NI].rearrange(
                "(t p) n -> p t n", p=P))
        for dc in range(CT):
            pp = mm_ps.tile([P, NI], F32, tag="mm")
            for ct in range(CT):
                nc.tensor.matmul(
                    pp, lhsT=m2t[:, ct, dc * P:(dc + 1) * P],
                    rhs=att_sb[:, ct, :],
                    start=(ct == 0), stop=(ct == CT - 1))
            ob = outs_pool.tile([P, NI], F32, tag="ob")
            nc.vector.tensor_mul(out=ob, in0=pp, in1=rec)
            nc.vector.tensor_scalar_add(out=ob, in0=ob,
                                        scalar1=w4[:, dc:dc + 1])
            nc.vector.tensor_add(out=ob, in0=ob, in1=xr[:, dc, :])
            nc.sync.dma_start(
                out=out_d[dc * P:(dc + 1) * P, icq * NI:(icq + 1) * NI], in_=ob)


def _build(repeat=1):
    nc = bacc.Bacc()
    x_d = nc.declare_dram_parameter("x", [C, N], F32, isOutput=False)
    w_d = {w: nc.declare_dram_parameter(w, [C, C], F32, isOutput=False)
           for w in WEIGHT_NAMES}
    p_d = {p: nc.declare_dram_parameter(p, [C], F32, isOutput=False)
           for p in PARAM_NAMES}
    out_d = nc.declare_dram_parameter("out", [C, NQ], F32, isOutput=True)
    with tile.TileContext(nc) as tc, ExitStack() as ctx:
        _emit(ctx, nc, tc, x_d, w_d, p_d, out_d, repeat=repeat)
    nc.finalize()
    return nc


def _get_nc():
    if "nc" not in _BUILD_CACHE:
        _BUILD_CACHE["nc"] = _build()
    return _BUILD_CACHE["nc"]


def _make_in_maps(x, gn_scale, gn_bias, wq, bq, wk, bk, wv, bv, wp, bp):
    xf = np.ascontiguousarray(np.asarray(x, dtype=np.float32).reshape(B, C, N))
    shared = {
        "wq": np.ascontiguousarray(np.asarray(wq, np.float32)),
        "wk": np.ascontiguousarray(np.asarray(wk, np.float32)),
        "wv": np.ascontiguousarray(np.asarray(wv, np.float32)),
        # wp ships pre-transposed: the kernel wants c_in on rows.
        "wp": np.ascontiguousarray(np.asarray(wp, np.float32).T),
        "bq": np.ascontiguousarray(np.asarray(bq, np.float32)),
        "bk": np.ascontiguousarray(np.asarray(bk, np.float32)),
        "bv": np.ascontiguousarray(np.asarray(bv, np.float32)),
        "bp": np.ascontiguousarray(np.asarray(bp, np.float32)),
        "gn_scale": np.ascontiguousarray(np.asarray(gn_scale, np.float32)),
        "gn_bias": np.ascontiguousarray(np.asarray(gn_bias, np.float32)),
    }
    in_maps = []
    for core in range(8):
        bi, qh = core // 2, core % 2
        xb = xf[bi]
        if qh == 0:
            xc = xb
        else:
            xc = np.ascontiguousarray(
                np.concatenate([xb[:, NQ:], xb[:, :NQ]], axis=1))
        in_maps.append({"x": xc, **shared})
    return in_maps


def _gather(results):
    out = np.empty((B, C, N), np.float32)
    for core in range(8):
        bi, qh = core // 2, core % 2
        out[bi, :, qh * NQ:(qh + 1) * NQ] = results[core]["out"]
    return out.reshape(B, C, HW, HW)


def kernel(x, gn_scale, gn_bias, wq, bq, wk, bk, wv, bv, wp, bp):
    nc = _get_nc()
    in_maps = _make_in_maps(x, gn_scale, gn_bias, wq, bq, wk, bk, wv, bv,
                            wp, bp)
    res = run_bass_kernel_spmd(nc, in_maps, core_ids=list(range(8)))
    return _gather(res.results)


# revision 62
# speedup vs baseline: 311.9651x; 311.9651x over previous
"""Trainium2 Bass kernel for an AttnBlock (GroupNorm -> QKV 1x1 conv ->
spatial self-attention -> output projection -> residual).

Full-input contract: kernel(**inputs) takes the unsharded numpy inputs and
returns the full (4, 512, 64, 64) float32 output.

Sharding: 8 cores = 4 batches x 2 query-halves. Each core group-norms its
batch, runs attention for its 2048 queries over all 4096 keys, and writes
its query-half of the output. The per-core x input is column-rotated on the
host so that each core's own queries are always columns [0, 2048) — this
keeps the SPMD program identical across cores.

Algebraic fusions (all exact up to rounding):
- scores: q_i.k_j = h_j^T (Wk^T Wq) h_i + (Wk^T bq).h_j + [terms constant
  in j, dropped: softmax over j is invariant]. So K is never materialized;
  S^T = H^T @ R with R = (Wk^T Wq)^T-weighted H_q, and the (Wk^T bq).h_j
  term enters as a per-partition bias of the exp activation.
- attention output: Wp @ (V P) = (Wp Wv) @ (H P) + Wp bv (softmax weights
  sum to 1), so V is never materialized either: A = H-space attention
  (lhsT = H^T blocks), projected by M2 = Wp Wv, plus w4 = Wp bv + bp.
- softmax skips the max-subtraction (scores are ~N(0,1) after the c^-0.5
  scale); denominators: E tiles are accumulated on DVE and reduced across
  partitions by a single all-ones fp32 matmul per query chunk, and divided
  out after the output projection.

Numerics: matmuls in bf16 with fp32 PSUM accumulation; statistics, softmax
denominators and the final combine in fp32.
"""

from contextlib import ExitStack

import numpy as np

import concourse.mybir as mybir
import concourse.tile as tile
from concourse import bacc
from concourse.bass_utils import run_bass_kernel_spmd

# Problem geometry (hardcoded; the grading harness stages only kernel.py).
B = 4
C = 512
HW = 64
N = HW * HW          # 4096 keys per batch
NQ = N // 2          # 2048 queries per core
GROUPS = 32
GSIZE = C // GROUPS  # 16 channels per group
EPS = 1e-6

P = 128
CT = C // P          # 4 channel chunks
JT = N // P          # 32 key chunks of 128
NI = 512             # free-dim tile (queries / keys / channels)
IC = NQ // NI        # 4 query chunks per core

F32 = mybir.dt.float32
BF16 = mybir.dt.bfloat16

PARAM_NAMES = ("bq", "bk", "bv", "bp", "gn_scale", "gn_bias")
WEIGHT_NAMES = ("wq", "wk", "wv", "wp")

_BUILD_CACHE = {}


def _emit(ctx, nc, tc, x_d, w_d, p_d, out_d, repeat=1):
    AF = mybir.ActivationFunctionType
    ALU = mybir.AluOpType

    consts = ctx.enter_context(tc.tile_pool(name="consts", bufs=1))
    small = ctx.enter_context(tc.tile_pool(name="small", bufs=4))
    stage = ctx.enter_context(tc.tile_pool(name="stage", bufs=6))
    big = ctx.enter_context(tc.tile_pool(name="big", bufs=2))
    rpool = ctx.enter_context(tc.tile_pool(name="rpool", bufs=1))
    wpool = ctx.enter_context(tc.tile_pool(name="wpool", bufs=1))
    epool = ctx.enter_context(tc.tile_pool(name="epool", bufs=6))
    attn_pool = ctx.enter_context(tc.tile_pool(name="attn_pool", bufs=2))
    outs_pool = ctx.enter_context(tc.tile_pool(name="outs_pool", bufs=3))
    mm_ps = ctx.enter_context(tc.tile_pool(name="mm_ps", bufs=4, space="PSUM"))
    acc_ps = ctx.enter_context(tc.tile_pool(name="acc_ps", bufs=4, space="PSUM"))

    for _rep in range(repeat):
        _emit_body(nc, tc, x_d, w_d, p_d, out_d, consts, small, stage, big,
                   rpool, wpool, epool, attn_pool, outs_pool, mm_ps, acc_ps,
                   AF, ALU, _rep)


def _emit_body(nc, tc, x_d, w_d, p_d, out_d, consts, small, stage, big,
               rpool, wpool, epool, attn_pool, outs_pool, mm_ps, acc_ps,
               AF, ALU, rep):
    # ---- constants -------------------------------------------------------
    # Pool-engine constants first: the hT transposes need `ident_bf` and
    # nothing should queue ahead of it on GpSimd.
    ident_bf = consts.tile([P, P], BF16, tag="ident_bf")
    nc.gpsimd.memset(ident_bf, 0.0)
    nc.gpsimd.affine_select(
        out=ident_bf, in_=ident_bf, compare_op=ALU.not_equal, fill=1.0,
        base=0, pattern=[[-1, P]], channel_multiplier=1,
    )
    ones_f = consts.tile([P, P], F32, tag="ones_f")
    nc.vector.memset(ones_f, 1.0)

    # Per-channel params as (128, CT): column cc = channels [cc*128, ..+128).
    # SWDGE (gpsimd) keeps these small gathers off the HWDGE queues that
    # stream x and the weights.
    par = {}
    for name in PARAM_NAMES:
        t = consts.tile([P, CT], F32, tag=f"par_{name}", name=f"par_{name}")
        nc.gpsimd.dma_start(out=t, in_=p_d[name][:].rearrange("(t p) -> p t", p=P))
        par[name] = t
    # Group-reduction matrices. G: (128, 8) with G[p, g] = 1/GSIZE iff
    # p // GSIZE == g. GE: (8, 128) with GE[g, p] = 1 iff p // GSIZE == g.
    GPC = P // GSIZE  # 8 groups per 128-channel chunk
    gmat = consts.tile([P, GPC], F32, tag="gmat")
    nc.gpsimd.memset(gmat, 1.0 / GSIZE)
    nc.gpsimd.affine_select(
        out=gmat, in_=gmat, compare_op=ALU.is_ge, fill=0.0,
        base=0, pattern=[[-GSIZE, GPC]], channel_multiplier=1,
    )
    nc.gpsimd.affine_select(
        out=gmat, in_=gmat, compare_op=ALU.is_ge, fill=0.0,
        base=GSIZE - 1, pattern=[[GSIZE, GPC]], channel_multiplier=-1,
    )
    gexp = consts.tile([GPC, P], F32, tag="gexp")
    nc.gpsimd.memset(gexp, 1.0)
    nc.gpsimd.affine_select(
        out=gexp, in_=gexp, compare_op=ALU.is_ge, fill=0.0,
        base=0, pattern=[[1, P]], channel_multiplier=-GSIZE,
    )
    nc.gpsimd.affine_select(
        out=gexp, in_=gexp, compare_op=ALU.is_ge, fill=0.0,
        base=GSIZE - 1, pattern=[[-1, P]], channel_multiplier=GSIZE,
    )
    eps8 = consts.tile([GPC, 1], F32, tag="eps8")
    nc.vector.memset(eps8, EPS)

    # ---- weights: one DMA + one bf16 cast per weight --------------------
    # The host ships "wp" already transposed (c_in on rows), so all four
    # arrive in the layout their matmuls need.
    w_nat = {}
    for wname in WEIGHT_NAMES:
        w_nat[wname] = wpool.tile([P, CT, C], BF16, tag=f"wn_{wname}",
                                  name=f"wn_{wname}")
        ws = stage.tile([P, CT, C], F32, tag="wstage",
                        name=f"ws_{rep}_{wname}", bufs=2)
        nc.sync.dma_start(
            out=ws, in_=w_d[wname][:].rearrange("(t p) c -> p t c", p=P))
        nc.vector.tensor_copy(out=w_nat[wname], in_=ws)
    wpT = w_nat["wp"]
    # bf16 bias casts (only needed by the w2/w4 fusions below)
    bq_bf = consts.tile([P, CT], BF16, tag="bq_bf")
    nc.vector.tensor_copy(out=bq_bf, in_=par["bq"])
    bv_bf = consts.tile([P, CT], BF16, tag="bv_bf")
    nc.vector.tensor_copy(out=bv_bf, in_=par["bv"])

    # ---- weight-only fusions (overlap with the x DMA / GroupNorm) --------
    # W3 = Wq^T Wk, stored (b=c_q partition-chunks, a=c_k free).
    w3 = wpool.tile([P, CT, C], BF16, tag="w3")
    for bt in range(CT):
        ps = mm_ps.tile([P, C], F32, tag="mm")
        for co in range(CT):
            nc.tensor.matmul(
                ps, lhsT=w_nat["wq"][:, co, bt * P:(bt + 1) * P],
                rhs=w_nat["wk"][:, co, :],
                start=(co == 0), stop=(co == CT - 1))
        nc.vector.tensor_copy(out=w3[:, bt, :], in_=ps)
    # M2T = (Wp Wv)^T, stored (a=c_attn partition-chunks, d=c_out free).
    m2t = wpool.tile([P, CT, C], BF16, tag="m2t")
    for at in range(CT):
        ps = mm_ps.tile([P, C], F32, tag="mm")
        for ec in range(CT):
            nc.tensor.matmul(
                ps, lhsT=w_nat["wv"][:, ec, at * P:(at + 1) * P],
                rhs=wpT[:, ec, :],
                start=(ec == 0), stop=(ec == CT - 1))
        nc.vector.tensor_copy(out=m2t[:, at, :], in_=ps)
    # w2 = Wk^T bq (bf16, used as a matmul operand against h).
    w2_bf = consts.tile([P, CT], BF16, tag="w2_bf")
    for at in range(CT):
        ps = mm_ps.tile([P, 1], F32, tag="mm")
        for co in range(CT):
            nc.tensor.matmul(
                ps, lhsT=w_nat["wk"][:, co, at * P:(at + 1) * P],
                rhs=bq_bf[:, co:co + 1],
                start=(co == 0), stop=(co == CT - 1))
        nc.vector.tensor_copy(out=w2_bf[:, at:at + 1], in_=ps)
    # w4 = Wp bv + bp (per output channel, f32).
    w4 = consts.tile([P, CT], F32, tag="w4")
    for dt_ in range(CT):
        ps = mm_ps.tile([P, 1], F32, tag="mm")
        for ec in range(CT):
            nc.tensor.matmul(
                ps, lhsT=wpT[:, ec, dt_ * P:(dt_ + 1) * P],
                rhs=bv_bf[:, ec:ec + 1],
                start=(ec == 0), stop=(ec == CT - 1))
        nc.vector.tensor_add(out=w4[:, dt_:dt_ + 1], in0=ps,
                             in1=par["bp"][:, dt_:dt_ + 1])

    # ---- x load + GroupNorm + normalize (to bf16 h) ----------------------
    h = big.tile([P, CT, N], BF16, tag="big")
    # hT blocks (keys on partitions), filled per channel chunk as h lands.
    ht = big.tile([P, JT, C], BF16, tag="big")
    for cc in range(CT):
        stats = small.tile([P, 8, 6], F32, tag="gn_stats",
                           name=f"gn_stats_{rep}_{cc}")
        xs = stage.tile([P, N], F32, tag="xstage", name=f"xs_{rep}_{cc}",
                        bufs=2)
        nc.sync.dma_start(out=xs, in_=x_d[cc * P:(cc + 1) * P, :])
        for sg in range(8):
            nc.vector.bn_stats(out=stats[:, sg, :],
                               in_=xs[:, sg * NI:(sg + 1) * NI])
        mv = small.tile([P, 2], F32, tag="gn_mv")
        nc.vector.bn_aggr(out=mv, in_=stats)
        # stat2 = [mean_c, E[x^2]_c];  E[x^2] = mean^2 + var in one op
        stat2 = small.tile([P, 2], F32, tag="gn_stat2")
        nc.vector.tensor_copy(out=stat2[:, 0:1], in_=mv[:, 0:1])
        nc.vector.tensor_scalar(
            out=stat2[:, 1:2], in0=mv[:, 0:1], scalar1=mv[:, 0:1],
            scalar2=mv[:, 1:2], op0=ALU.mult, op1=ALU.add)
        # group-combine on PE: (8, 2) = G^T @ stat2
        g_ps = acc_ps.tile([GPC, 2], F32, tag="acc")
        nc.tensor.matmul(g_ps, lhsT=gmat, rhs=stat2, start=True, stop=True)
        g_sb = small.tile([GPC, 2], F32, tag="gn_gsb")
        nc.vector.tensor_copy(out=g_sb, in_=g_ps)
        # grp = [mean_g, rstd_g];  rstd via sqrt(-1*(mean^2 - E2) + eps)
        grp = small.tile([GPC, 2], F32, tag="gn_grp")
        nc.vector.tensor_copy(out=grp[:, 0:1], in_=g_sb[:, 0:1])
        nvar = small.tile([GPC, 1], F32, tag="gn_nvar")
        nc.vector.tensor_scalar(
            out=nvar, in0=g_sb[:, 0:1], scalar1=g_sb[:, 0:1],
            scalar2=g_sb[:, 1:2], op0=ALU.mult, op1=ALU.subtract)
        sd = small.tile([GPC, 1], F32, tag="gn_sd")
        nc.scalar.activation(out=sd, in_=nvar, func=AF.Sqrt, bias=eps8,
                             scale=-1.0)
        nc.vector.reciprocal(out=grp[:, 1:2], in_=sd)
        # expand back to per-channel via PE: (128, 2) = GE^T @ grp
        e_ps = acc_ps.tile([P, 2], F32, tag="acc")
        nc.tensor.matmul(e_ps, lhsT=gexp, rhs=grp, start=True, stop=True)
        e_sb = small.tile([P, 2], F32, tag="gn_esb")
        nc.vector.tensor_copy(out=e_sb, in_=e_ps)
        # a_c = gn_scale * rstd ; b_c = gn_bias - mean * a_c
        a_c = small.tile([P, 1], F32, tag="gn_a")
        nc.vector.tensor_mul(out=a_c, in0=par["gn_scale"][:, cc:cc + 1],
                             in1=e_sb[:, 1:2])
        nb_c = small.tile([P, 1], F32, tag="gn_nb")
        nc.vector.tensor_scalar(
            out=nb_c, in0=e_sb[:, 0:1], scalar1=a_c,
            scalar2=par["gn_bias"][:, cc:cc + 1],
            op0=ALU.mult, op1=ALU.subtract)
        # b_c for the ACT half (needs the true sign)
        b_c = small.tile([P, 1], F32, tag="gn_b")
        nc.vector.tensor_scalar_mul(out=b_c, in0=nb_c, scalar1=-1.0)
        # h = a_c * x - nb_c, split across DVE and ACT halves
        nc.vector.tensor_scalar(
            out=h[:, cc, :N // 2], in0=xs[:, :N // 2], scalar1=a_c,
            scalar2=nb_c, op0=ALU.mult, op1=ALU.subtract)
        nc.scalar.activation(
            out=h[:, cc, N // 2:], in_=xs[:, N // 2:], func=AF.Identity,
            scale=a_c, bias=b_c)
        # hT blocks for this channel chunk: 4 transposes packed per PSUM
        # bank (disjoint column ranges), one strided eviction per pack.
        for jg in range(JT // 4):
            tp = acc_ps.tile([P, 4, P], BF16, tag="acc",
                             name=f"htp_{rep}_{cc}_{jg}")
            for k in range(4):
                jc = jg * 4 + k
                nc.tensor.matmul(
                    tp[:, k, :], lhsT=h[:, cc, jc * P:(jc + 1) * P],
                    rhs=ident_bf, is_transpose=True, skip_group_check=True)
            dst = ht[:, jg * 4:(jg + 1) * 4, cc * P:(cc + 1) * P]
            if jg % 2 == 0:
                nc.vector.tensor_copy(out=dst, in_=tp)
            else:
                nc.scalar.activation(out=dst, in_=tp, func=AF.Identity)

    # ---- h-derived operands ---------------------------------------------
    inv_sqrt_c = float(C) ** -0.5
    # R = (Wk^T Wq)^T-weighted H_q: R[a, i] = sum_b W3[b, a] h[b, i].
    # icq-major so attention on the first query chunk can start early.
    r_sb = rpool.tile([P, CT, NQ], BF16, tag="r")
    for icq in range(IC):
        for at in range(CT):
            ps = mm_ps.tile([P, NI], F32, tag="mm")
            for bc in range(CT):
                nc.tensor.matmul(
                    ps, lhsT=w3[:, bc, at * P:(at + 1) * P],
                    rhs=h[:, bc, icq * NI:(icq + 1) * NI],
                    start=(bc == 0), stop=(bc == CT - 1))
            nc.vector.tensor_copy(out=r_sb[:, at, icq * NI:(icq + 1) * NI],
                                  in_=ps)
    # r2[j] = (Wk^T bq) . h_j, scaled by c^-0.5: per-partition exp bias.
    # 8 j-chunks pack into one PSUM bank (disjoint f32 columns).
    r2s = consts.tile([P, JT], F32, tag="r2s")
    for jg in range(JT // 8):
        ps = acc_ps.tile([P, 8], F32, tag="acc", name=f"r2p_{rep}_{jg}")
        for k in range(8):
            jc = jg * 8 + k
            for ac in range(CT):
                nc.tensor.matmul(
                    ps[:, k:k + 1], lhsT=h[:, ac, jc * P:(jc + 1) * P],
                    rhs=w2_bf[:, ac:ac + 1],
                    start=(ac == 0), stop=(ac == CT - 1),
                    skip_group_check=True)
        nc.vector.tensor_scalar_mul(out=r2s[:, jg * 8:(jg + 1) * 8], in0=ps,
                                    scalar1=inv_sqrt_c)

    # ---- attention + output projection + residual ------------------------
    for icq in range(IC):
        att_ps = [acc_ps.tile([P, NI], F32, tag="acc",
                              name=f"att_ps_{rep}_{icq}_{ct}")
                  for ct in range(CT)]
        e_sum = outs_pool.tile([P, NI], F32, tag="esum", bufs=2,
                                name=f"esum_{rep}_{icq}")
        for jc in range(JT):
            s_ps = mm_ps.tile([P, NI], F32, tag="mm")
            for ac in range(CT):
                nc.tensor.matmul(
                    s_ps, lhsT=h[:, ac, jc * P:(jc + 1) * P],
                    rhs=r_sb[:, ac, icq * NI:(icq + 1) * NI],
                    start=(ac == 0), stop=(ac == CT - 1))
            e = epool.tile([P, NI], BF16, tag="e")
            nc.scalar.activation(out=e, in_=s_ps, func=AF.Exp,
                                 scale=inv_sqrt_c, bias=r2s[:, jc:jc + 1])
            for ct in range(CT):
                nc.tensor.matmul(
                    att_ps[ct], lhsT=ht[:, jc, ct * P:(ct + 1) * P], rhs=e,
                    start=(jc == 0), stop=(jc == JT - 1))
            if jc == 0:
                nc.vector.tensor_copy(out=e_sum, in_=e)
            else:
                nc.vector.tensor_add(out=e_sum, in0=e_sum, in1=e)
        den_ps = mm_ps.tile([P, NI], F32, tag="mm",
                            name=f"den_ps_{rep}_{icq}")
        nc.tensor.matmul(den_ps, lhsT=ones_f, rhs=e_sum, start=True, stop=True)
        rec = outs_pool.tile([P, NI], F32, tag="rec", bufs=2,
                              name=f"rec_{rep}_{icq}")
        nc.vector.reciprocal(out=rec, in_=den_ps)
        att_sb = attn_pool.tile([P, CT, NI], BF16, tag="attn")
        for ct in range(CT):
            nc.vector.tensor_copy(out=att_sb[:, ct, :], in_=att_ps[ct])
        xr = outs_pool.tile([P, CT, NI], F32, tag="xres", bufs=2,
                            name=f"xr_{rep}_{icq}")
        nc.sync.dma_start(
            out=xr, in_=x_d[:, icq * NI:(icq + 1) * NI].rearrange(
                "(t p) n -> p t n", p=P))
        for dc in range(CT):
            pp = mm_ps.tile([P, NI], F32, tag="mm")
            for ct in range(CT):
                nc.tensor.matmul(
                    pp, lhsT=m2t[:, ct, dc * P:(dc + 1) * P],
                    rhs=att_sb[:, ct, :],
                    start=(ct == 0), stop=(ct == CT - 1))
            ob = outs_pool.tile([P, NI], F32, tag="ob")
            nc.vector.tensor_mul(out=ob, in0=pp, in1=rec)
            nc.vector.tensor_scalar_add(out=ob, in0=ob,
                                        scalar1=w4[:, dc:dc + 1])
            nc.vector.tensor_add(out=ob, in0=ob, in1=xr[:, dc, :])
            nc.sync.dma_start(
                out=out_d[dc * P:(dc + 1) * P, icq * NI:(icq + 1) * NI], in_=ob)


def _build(repeat=1):
    nc = bacc.Bacc()
    x_d = nc.declare_dram_parameter("x", [C, N], F32, isOutput=False)
    w_d = {w: nc.declare_dram_parameter(w, [C, C], F32, isOutput=False)
           for w in WEIGHT_NAMES}
    p_d = {p: nc.declare_dram_parameter(p, [C], F32, isOutput=False)
           for p in PARAM_NAMES}
    out_d = nc.declare_dram_parameter("out", [C, NQ], F32, isOutput=True)
    with tile.TileContext(nc) as tc, ExitStack() as ctx:
        _emit(ctx, nc, tc, x_d, w_d, p_d, out_d, repeat=repeat)
    nc.finalize()
    return nc


def _get_nc():
    if "nc" not in _BUILD_CACHE:
        _BUILD_CACHE["nc"] = _build()
    return _BUILD_CACHE["nc"]


def _make_in_maps(x, gn_scale, gn_bias, wq, bq, wk, bk, wv, bv, wp, bp):
    xf = np.ascontiguousarray(np.asarray(x, dtype=np.float32).reshape(B, C, N))
    shared = {
        "wq": np.ascontiguousarray(np.asarray(wq, np.float32)),
        "wk": np.ascontiguousarray(np.asarray(wk, np.float32)),
        "wv": np.ascontiguousarray(np.asarray(wv, np.float32)),
        # wp ships pre-transposed: the kernel wants c_in on rows.
        "wp": np.ascontiguousarray(np.asarray(wp, np.float32).T),
        "bq": np.ascontiguousarray(np.asarray(bq, np.float32)),
        "bk": np.ascontiguousarray(np.asarray(bk, np.float32)),
        "bv": np.ascontiguousarray(np.asarray(bv, np.float32)),
        "bp": np.ascontiguousarray(np.asarray(bp, np.float32)),
        "gn_scale": np.ascontiguousarray(np.asarray(gn_scale, np.float32)),
        "gn_bias": np.ascontiguousarray(np.asarray(gn_bias, np.float32)),
    }
    in_maps = []
    for core in range(8):
        bi, qh = core // 2, core % 2
        xb = xf[bi]
        if qh == 0:
            xc = xb
        else:
            xc = np.ascontiguousarray(
                np.concatenate([xb[:, NQ:], xb[:, :NQ]], axis=1))
        in_maps.append({"x": xc, **shared})
    return in_maps


def _gather(results):
    out = np.empty((B, C, N), np.float32)
    for core in range(8):
        bi, qh = core // 2, core % 2
        out[bi, :, qh * NQ:(qh + 1) * NQ] = results[core]["out"]
    return out.reshape(B, C, HW, HW)


def kernel(x, gn_scale, gn_bias, wq, bq, wk, bk, wv, bv, wp, bp):
    nc = _get_nc()
    in_maps = _make_in_maps(x, gn_scale, gn_bias, wq, bq, wk, bk, wv, bv,
                            wp, bp)
    res = run_bass_kernel_spmd(nc, in_maps, core_ids=list(range(8)))
    return _gather(res.results)
